# revision 19
# baseline (speedup 1.0000x reference)
# Grouped GRU layer on 8 Trainium2 NeuronCores (one group per core).
#
# Problem: x [64, 500, 1024], 8 independent groups of (IG=128 -> HG=128) GRUs.
#   xp = einsum('btgi,gji->btgj', xg, W_ih) + b_ih        (input projection)
#   per step: hp = h @ W_hh[g].T + b_hh
#             r = sig(xr+hr); z = sig(xz+hz); n = tanh(xn + r*hn)
#             h = (1-z)*n + z*h
#
# Sharding: group g -> core g. Per-core layout fully "transposed":
#   state h^T [HG=128 partitions, B=64 free], weights pre-transposed on host.
#
# The per-step serial dependency chain is the wall (T=500 steps); the kernel
# minimizes the number and cost of chained engine visits per step:
#   sigmoid(r) [ACT] -> scan [DVE] -> tanh [ACT] -> w=omz*n [DVE] -> matmul
# - bf16 recurrent matmuls (1 PE cycle/row instead of 4)
# - W_z/b_z negated on host: sigmoid directly yields omz = 1-z, and the
#   post-tanh multiply w = omz*n has both producers on ACT (single semaphore,
#   engine-level wait)
# - state fed to PE as three parts h = h_prev + q + w (q = -omz*h_prev), so
#   only w is tanh-dependent
# - u = r*(hn+b_hhn) and n_arg = u + xn fused into ONE tensor_tensor_scan
#   over interleaved lanes: state(2b) = hn_b, state(2b+1) = r_b*hn_b + xn_b;
#   hn (+b_hhn) and xn are staged into the interleaved buffer off-chain
# - xn chunk copies (with b_ihn folded) scheduled into ACT queue slack

import numpy as np

B, T, IN, HID, G = 64, 500, 1024, 1024, 8
IG, HG = 128, 128

PSUM_STEPS = 8          # steps per PSUM bank chunk ([128, 8*64] fp32 = 1 bank)
RING_STEPS = 50         # output ring buffer length (steps) per DMA-out chunk

_CACHE = {}


def _build_program():
    import concourse.tile as tile
    from concourse import bacc, mybir

    f32 = mybir.dt.float32
    bf16 = mybir.dt.bfloat16
    AF = mybir.ActivationFunctionType
    ALU = mybir.AluOpType

    nc = bacc.Bacc()
    xT = nc.declare_dram_parameter("xT", [IG, T * B], bf16, isOutput=False)
    wih = nc.declare_dram_parameter("wih", [IG, 3 * HG], bf16, isOutput=False)
    whh = nc.declare_dram_parameter("whh", [HG, 3 * HG], bf16, isOutput=False)
    # per-partition bias columns: [r_bias, -z_bias, b_ihn, b_hhn]
    biases = nc.declare_dram_parameter("biases", [HG, 4], f32, isOutput=False)
    y = nc.declare_dram_parameter("y", [HG, T * B], f32, isOutput=True)

    from contextlib import ExitStack

    with tile.TileContext(nc) as tc, ExitStack() as ctx:
        consts = ctx.enter_context(tc.tile_pool(name="consts", bufs=1))
        xpool = ctx.enter_context(tc.tile_pool(name="xin", bufs=3))
        pr_pool = ctx.enter_context(tc.tile_pool(name="pr", bufs=2, space="PSUM"))
        pz_pool = ctx.enter_context(tc.tile_pool(name="pz", bufs=2, space="PSUM"))
        pn_pool = ctx.enter_context(tc.tile_pool(name="pn", bufs=2, space="PSUM"))
        hp_pool = ctx.enter_context(tc.tile_pool(name="hpn", bufs=2, space="PSUM"))
        work = ctx.enter_context(tc.tile_pool(name="work", bufs=4))
        ring_pool = ctx.enter_context(tc.tile_pool(name="ring", bufs=2))

        w_ih = consts.tile([IG, 3 * HG], bf16)
        w_hh = consts.tile([HG, 3 * HG], bf16)
        bias4 = consts.tile([HG, 4], f32)
        nc.sync.dma_start(out=w_ih, in_=wih[:, :])
        nc.sync.dma_start(out=w_hh, in_=whh[:, :])
        nc.sync.dma_start(out=bias4, in_=biases[:, :])
        bias_r = bias4[:, 0:1]
        bias_zn = bias4[:, 1:2]   # negated z bias (W_z also negated on host)
        b_ihn = bias4[:, 2:3]
        b_hhn = bias4[:, 3:4]

        zeros_sb = consts.tile([HG, B], f32)
        nc.vector.memset(zeros_sb, 0.0)

        n_chunks = (T + PSUM_STEPS - 1) // PSUM_STEPS
        CB = PSUM_STEPS * B

        # Interleaved scan operand buffers, one pair per chunk parity.
        # mix0: even lanes 0 (reset state to hn), odd lanes r (sigmoid out).
        # mix1: even lanes hn+b_hhn (staged per step), odd lanes xn+b_ihn
        # (chunk copy). Even lanes of mix0 are memset once and never touched.
        mix0 = [consts.tile([HG, 2 * CB], f32, name=f"mix0_{i}") for i in range(2)]
        mix1 = [consts.tile([HG, 2 * CB], f32, name=f"mix1_{i}") for i in range(2)]
        for mt in mix0 + mix1:
            nc.vector.memset(mt, 0.0)
        # t=0: hn lanes of slice 0 must hold plain b_hhn (hpn(0) = 0)
        nc.scalar.activation(mix1[0][:, 0:2 * B:2], zeros_sb,
                             AF.Identity, bias=b_hhn)

        h_prev = zeros_sb
        ring = None
        nxt = {}

        def start_chunk(c):
            """DMA + input projections for chunk c."""
            t0 = c * PSUM_STEPS
            steps = min(PSUM_STEPS, T - t0)
            nb = steps * B
            d = {"steps": steps, "t0": t0, "par": c % 2}
            x_c = xpool.tile([IG, CB], bf16, tag="xc")
            nc.sync.dma_start(out=x_c[:, :nb], in_=xT[:, t0 * B : t0 * B + nb])
            p_r = pr_pool.tile([HG, CB], f32, tag="pr")
            p_z = pz_pool.tile([HG, CB], f32, tag="pz")
            p_n = pn_pool.tile([HG, CB], f32, tag="pn")
            nc.tensor.matmul(p_r[:, :nb], w_ih[:, 0:HG], x_c[:, :nb],
                             start=True, stop=False, skip_group_check=True)
            nc.tensor.matmul(p_z[:, :nb], w_ih[:, HG:2 * HG], x_c[:, :nb],
                             start=True, stop=False, skip_group_check=True)
            nc.tensor.matmul(p_n[:, :nb], w_ih[:, 2 * HG:3 * HG], x_c[:, :nb],
                             start=True, stop=True, skip_group_check=True)
            d["p_r"], d["p_z"], d["p_n"] = p_r, p_z, p_n
            return d

        def copy_half_xn(d, half):
            # xn -> odd lanes of mix1 with b_ihn folded in
            nb = d["steps"] * B
            lo = half * (CB // 2)
            hi = min((half + 1) * (CB // 2), nb)
            if lo < hi:
                m1 = mix1[d["par"]]
                nc.scalar.activation(m1[:, 2 * lo + 1 : 2 * hi : 2],
                                     d["p_n"][:, lo:hi],
                                     AF.Identity, bias=b_ihn)

        cur = start_chunk(0)
        copy_half_xn(cur, 0)
        copy_half_xn(cur, 1)

        for c in range(n_chunks):
            steps = cur["steps"]
            t0 = cur["t0"]
            p_r, p_z = cur["p_r"], cur["p_z"]
            m0c, m1c = mix0[cur["par"]], mix1[cur["par"]]

            for s in range(steps):
                t = t0 + s
                sl = slice(s * B, (s + 1) * B)
                msl = slice(2 * s * B, 2 * (s + 1) * B)
                if t % RING_STEPS == 0:
                    ring = ring_pool.tile([HG, RING_STEPS * B], f32, tag="ring")
                rsl = slice((t % RING_STEPS) * B, (t % RING_STEPS + 1) * B)

                # --- step t gates (pre-activations in PSUM) ---
                # sigmoid(r) into the odd lanes of mix0
                nc.scalar.activation(m0c[:, 2 * s * B + 1 : 2 * (s + 1) * B : 2],
                                     p_r[:, sl], AF.Sigmoid, bias=bias_r)

                # fused u+n_arg: state(2b) = hn_b, state(2b+1) = r_b*hn_b+xn_b
                sc = work.tile([HG, 2 * B], f32, tag="sc")
                nc.vector.tensor_tensor_scan(sc, m0c[:, msl], m1c[:, msl],
                                             0.0, ALU.mult, ALU.add)

                # omz = 1 - z = sigmoid(-a_z)
                omz = work.tile([HG, B], bf16, tag="omz")
                nc.scalar.activation(omz, p_z[:, sl], AF.Sigmoid, bias=bias_zn)
                # q = -omz * h_prev   (part 2 of next state; off chain)
                q_sb = work.tile([HG, B], bf16, tag="q")
                nc.vector.scalar_tensor_tensor(
                    out=q_sb, in0=omz, scalar=-1.0, in1=h_prev,
                    op0=ALU.mult, op1=ALU.mult)

                n_sb = work.tile([HG, B], bf16, tag="n")
                nc.scalar.activation(n_sb, sc[:, 1::2], AF.Tanh)
                # w = omz * n   (the only post-tanh chain op)
                w_sb = work.tile([HG, B], bf16, tag="w")
                nc.vector.tensor_mul(w_sb, n_sb, omz)

                # h_new = h_prev + q + w. h_new on DVE so that w's tile has no
                # Pool reader (keeps w's attached wait on the tanh semaphore).
                w1 = work.tile([HG, B], f32, tag="w1")
                nc.gpsimd.tensor_add(w1, h_prev if t > 0 else zeros_sb, q_sb)
                h_new = ring[:, rsl]
                nc.vector.tensor_add(h_new, w1, w_sb)
                # bf16 copy of h for the next step's PE rhs
                h_bf = work.tile([HG, B], bf16, tag="hbf")
                nc.gpsimd.tensor_scalar_add(h_bf, h_new, 0.0)

                # --- recurrence matmuls feeding step t+1 ---
                if t + 1 < T:
                    in_this = s + 1 < steps
                    if in_this:
                        sl1 = slice((s + 1) * B, (s + 2) * B)
                        t_r, t_z = p_r[:, sl1], p_z[:, sl1]
                        m1n = m1c
                        e0 = 2 * (s + 1) * B
                    else:
                        nxt = start_chunk(c + 1)
                        t_r, t_z = nxt["p_r"][:, 0:B], nxt["p_z"][:, 0:B]
                        m1n = mix1[nxt["par"]]
                        e0 = 0
                    hpn = hp_pool.tile([HG, B], f32, tag="hpn")
                    # h_prev-part (ready at h_bf of step t-1; earliest)
                    if t > 0:
                        nc.tensor.matmul(t_r, w_hh[:, 0:HG], h_bf_prev,
                                         start=False, stop=False,
                                         skip_group_check=True)
                        nc.tensor.matmul(t_z, w_hh[:, HG:2 * HG], h_bf_prev,
                                         start=False, stop=False,
                                         skip_group_check=True)
                        nc.tensor.matmul(hpn, w_hh[:, 2 * HG:3 * HG], h_bf_prev,
                                         start=True, stop=False,
                                         skip_group_check=True)
                    # q-part (ready after omz, mid-step)
                    nc.tensor.matmul(t_r, w_hh[:, 0:HG], q_sb,
                                     start=False, stop=False,
                                     skip_group_check=True)
                    nc.tensor.matmul(t_z, w_hh[:, HG:2 * HG], q_sb,
                                     start=False, stop=False,
                                     skip_group_check=True)
                    nc.tensor.matmul(hpn, w_hh[:, 2 * HG:3 * HG], q_sb,
                                     start=(t == 0), stop=False,
                                     skip_group_check=True)
                    # w-part: r-gate first (unblocks sigmoid), then n, then z
                    nc.tensor.matmul(t_r, w_hh[:, 0:HG], w_sb,
                                     start=False, stop=True,
                                     skip_group_check=True)
                    nc.tensor.matmul(hpn, w_hh[:, 2 * HG:3 * HG], w_sb,
                                     start=False, stop=True,
                                     skip_group_check=True)
                    nc.tensor.matmul(t_z, w_hh[:, HG:2 * HG], w_sb,
                                     start=False, stop=True,
                                     skip_group_check=True)
                    # stage hn+b_hhn into the even lanes of the next step's
                    # scan buffer
                    nc.vector.tensor_scalar_add(
                        m1n[:, e0 : e0 + 2 * B : 2], hpn, b_hhn)

                h_prev = h_new
                h_bf_prev = h_bf

                if s == 1 and c > 0:
                    copy_half_xn(cur, 1)

                if (t + 1) % RING_STEPS == 0:
                    base = (t + 1 - RING_STEPS) * B
                    nc.sync.dma_start(out=y[:, base : base + RING_STEPS * B],
                                      in_=ring)

            if c + 1 < n_chunks:
                copy_half_xn(nxt, 0)
                cur = nxt
    nc.finalize()
    return nc


def _get_program():
    if "nc" not in _CACHE:
        _CACHE["nc"] = _build_program()
    return _CACHE["nc"]


def _prep_inputs(x, W_ih, W_hh, b_ih, b_hh):
    import ml_dtypes

    bf = ml_dtypes.bfloat16
    x = np.asarray(x, dtype=np.float32)
    W_ih = np.asarray(W_ih, dtype=np.float32)
    W_hh = np.asarray(W_hh, dtype=np.float32)
    b_ih = np.asarray(b_ih, dtype=np.float32)
    b_hh = np.asarray(b_hh, dtype=np.float32)

    # x [B,T,IN] -> per group [IG, T*B] with free index = t*B + b
    xg = x.reshape(B, T, G, IG)
    xT = np.ascontiguousarray(np.transpose(xg, (2, 3, 1, 0))).reshape(G, IG, T * B)

    wihT = np.transpose(W_ih, (0, 2, 1)).copy()  # [G, IG, 3HG]
    whhT = np.transpose(W_hh, (0, 2, 1)).copy()  # [G, HG, 3HG]
    # negate the z-gate weights so sigmoid yields omz = 1-z directly
    wihT[:, :, HG:2 * HG] *= -1.0
    whhT[:, :, HG:2 * HG] *= -1.0

    biases = np.empty((G, HG, 4), np.float32)
    biases[:, :, 0] = b_ih[:, 0:HG] + b_hh[:, 0:HG]              # r
    biases[:, :, 1] = -(b_ih[:, HG:2 * HG] + b_hh[:, HG:2 * HG])  # -z
    biases[:, :, 2] = b_ih[:, 2 * HG:3 * HG]                      # b_ihn
    biases[:, :, 3] = b_hh[:, 2 * HG:3 * HG]                      # b_hhn

    in_maps = []
    for g in range(G):
        in_maps.append({
            "xT": xT[g].astype(bf),
            "wih": wihT[g].astype(bf),
            "whh": whhT[g].astype(bf),
            "biases": biases[g],
        })
    return in_maps


def _assemble(results):
    out = np.empty((B, T, HID), np.float32)
    for g in range(G):
        yg = results[g]["y"].reshape(HG, T, B)          # [h, t, b]
        out[:, :, g * HG:(g + 1) * HG] = np.transpose(yg, (2, 1, 0))
    return out


def run(x, W_ih, W_hh, b_ih, b_hh, trace=False):
    from concourse.bass_utils import run_bass_kernel_spmd

    nc = _get_program()
    in_maps = _prep_inputs(x, W_ih, W_hh, b_ih, b_hh)
    res = run_bass_kernel_spmd(nc, in_maps, list(range(G)), trace=trace)
    return _assemble(res.results), res


def kernel(x, W_ih, W_hh, b_ih, b_hh):
    out, _ = run(x, W_ih, W_hh, b_ih, b_hh)
    return out
